# revision 27
# baseline (speedup 1.0000x reference)
"""ATSS assignment kernel for Trainium2 (8 NeuronCores, data-parallel over batch).

Pipeline per core (2 images per core):
  1. PE matmul computes approximate scores u = g.p - |p|^2/2 for all
     (gt, pred) pairs (ranking-equivalent to negated squared distance).
  2. DVE tensor_reduce takes per-16-chunk maxima straight out of PSUM.
  3. A max8/max_index/match_replace cascade selects the top-72 chunks per
     GT row (contains all true top-64 preds: any chunk holding a top-64
     element must rank <= 64 by chunk-min).
  4. Per-chunk 256B blocks [cx|cy|w|h] are fetched by 72 single-offset
     indirect DMAs (one offset per partition each), exact fp32 squared
     distances are recomputed, and a second cascade extracts the exact
     top-64 (jax top_k tie semantics: ascending dist, lower index first;
     max_index returns first occurrence of duplicates).
  5. The IoU / mean+std threshold / center-inside epilogue runs in compact
     candidate space using the selection mask (cascade leftovers == NEG);
     rank-ordered outputs are produced by local_scatter through an
     inverted rank map.
"""

import sys

import numpy as np

if "/opt/trn_rl_repo" not in sys.path:
    sys.path.insert(0, "/opt/trn_rl_repo")

import concourse.bass as bass
import concourse.mybir as mybir
import concourse.tile as tile
from concourse import bacc
from concourse.bass import IndirectOffsetOnAxis

F32 = mybir.dt.float32
F32R = mybir.dt.float32r
U8 = mybir.dt.uint8
U16 = mybir.dt.uint16
I16 = mybir.dt.int16
U32 = mybir.dt.uint32
I32 = mybir.dt.int32
AL = mybir.AluOpType
ACT = mybir.ActivationFunctionType
AX = mybir.AxisListType

B, N, G, K = 16, 16384, 128, 64
NCORES = 8
BPC = B // NCORES          # batches (images) per core
CW = 16                    # chunk width for the prefilter
NCH = N // CW              # 1024 chunks per row
NSEL = 72                  # chunks kept per row (>= worst-case 65 + margin)
NIT1 = NSEL // 8           # cascade-1 iterations
CAND = NSEL * CW           # 1152 candidate preds per row
NIT2 = K // 8              # cascade-2 iterations
MMF = 512                  # matmul free-dim chunk (one PSUM bank)
NEG = -1e30


def build_program(num_devices=NCORES, debug_taps=False, stop_after=99):
    nc = bacc.Bacc(
        "TRN2",
        debug=False,
        target_bir_lowering=False,
        num_devices=num_devices,
    )
    ct_in = nc.dram_tensor("ct_in", [BPC, 2, G + N], F32, kind="ExternalInput")
    pred_cc = nc.dram_tensor("pred_cc", [BPC, NCH, 4, CW], F32, kind="ExternalInput")
    gt_boxes = nc.dram_tensor("gt_boxes", [BPC, G, 4], F32, kind="ExternalInput")
    out_ious = nc.dram_tensor("out_ious", [BPC, G, K], F32, kind="ExternalOutput")
    out_mask = nc.dram_tensor("out_mask", [BPC, G, K], U8, kind="ExternalOutput")
    out_kidx = nc.dram_tensor("out_kidx", [BPC, G, K], I32, kind="ExternalOutput")

    taps = None
    if debug_taps:
        taps = {
            "dbg_m16": nc.dram_tensor("dbg_m16", [BPC, G, NCH], F32, kind="ExternalOutput"),
            "dbg_ci16": nc.dram_tensor("dbg_ci16", [BPC, G, NSEL], U16, kind="ExternalOutput"),
            "dbg_cand": nc.dram_tensor("dbg_cand", [BPC, G, NSEL, 4 * CW], F32, kind="ExternalOutput"),
            "dbg_uc": nc.dram_tensor("dbg_uc", [BPC, G, CAND], F32, kind="ExternalOutput"),
            "dbg_pos": nc.dram_tensor("dbg_pos", [BPC, G, K], U16, kind="ExternalOutput"),
            "dbg_sel": nc.dram_tensor("dbg_sel", [BPC, G, CAND], F32, kind="ExternalOutput"),
            "dbg_iouc": nc.dram_tensor("dbg_iouc", [BPC, G, CAND], F32, kind="ExternalOutput"),
            "dbg_oidx": nc.dram_tensor("dbg_oidx", [BPC, G, CAND], U16, kind="ExternalOutput"),
            "dbg_rkm": nc.dram_tensor("dbg_rkm", [BPC, G, CAND], I16, kind="ExternalOutput"),
        }

    with tile.TileContext(nc) as tc:
        _emit(nc, tc, ct_in, pred_cc, gt_boxes, out_ious, out_mask,
              out_kidx, taps, stop_after)
    nc.compile()
    return nc


def _emit(nc, tc, ct_in, pred_cc, gt_boxes, out_ious, out_mask, out_kidx,
          taps=None, stop_after=99):
    cc_rows = pred_cc.ap().rearrange("b h c w -> (b h) (c w)")

    with (
        tc.tile_pool(name="const", bufs=1) as cpool,
        tc.tile_pool(name="sb", bufs=2) as sb,
        tc.tile_pool(name="cw", bufs=1) as cw,
        tc.tile_pool(name="rhsp", bufs=1) as rhsp,
        tc.tile_pool(name="ps0", bufs=4, space="PSUM") as ps0,
        tc.tile_pool(name="ps1", bufs=4, space="PSUM") as ps1,
    ):
        # constant -0.5 weights for the -(px^2+py^2)/2 accumulation matmul
        negh = cpool.tile([2, G], F32)
        # iota 0..15 repeated per chunk slot: j%16
        io_e = cpool.tile([G, CAND], U16)
        nc.gpsimd.iota(io_e, pattern=[[0, NSEL], [1, CW]], base=0,
                       channel_multiplier=0)
        # ranks 1..64 (scattered through pos -> rank map)
        io_r = cpool.tile([G, K], I16)
        nc.gpsimd.iota(io_r, pattern=[[1, K]], base=1, channel_multiplier=0)

        per_b = [dict() for _ in range(BPC)]
        for b in range(BPC):
            # ---------------- GT prep ----------------
            gt_sb = sb.tile([G, 4], F32)
            nc.sync.dma_start(gt_sb, gt_boxes.ap()[b])
            gx = gt_sb[:, 0:1]
            gy = gt_sb[:, 1:2]

            # GT xyxy + area ([G, 1] scalars)
            ghw = sb.tile([G, 1], F32, tag="ghw")
            nc.vector.tensor_scalar_mul(ghw, gt_sb[:, 2:3], 0.5)
            ghh = sb.tile([G, 1], F32, tag="ghh")
            nc.vector.tensor_scalar_mul(ghh, gt_sb[:, 3:4], 0.5)
            x1 = sb.tile([G, 1], F32, tag="x1")
            nc.vector.tensor_sub(x1, gx, ghw)
            x2 = sb.tile([G, 1], F32, tag="x2")
            nc.vector.tensor_add(x2, gx, ghw)
            y1 = sb.tile([G, 1], F32, tag="y1")
            nc.vector.tensor_sub(y1, gy, ghh)
            y2 = sb.tile([G, 1], F32, tag="y2")
            nc.vector.tensor_add(y2, gy, ghh)
            aw = sb.tile([G, 1], F32, tag="aw")
            nc.vector.tensor_sub(aw, x2, x1)
            ah = sb.tile([G, 1], F32, tag="ah")
            nc.vector.tensor_sub(ah, y2, y1)
            area_a = sb.tile([G, 1], F32, tag="area_a")
            nc.vector.tensor_mul(area_a, aw, ah)

            # stage row c = [gt_c (G) | pred_c (N)]; one DMA feeds both the
            # matmul weights (gt centers) and the moving preds.
            stage = rhsp.tile([2, G + N], F32)
            nc.sync.dma_start(stage, ct_in.ap()[b])
            if b == 0:
                nc.scalar.activation(negh, stage[:, 0:G], ACT.Copy,
                                     bias=-0.5, scale=0.0)

            # ---------------- scores + chunk maxima ----------------
            m16 = sb.tile([G, NCH], F32)
            psp = ps0 if b % 2 == 0 else ps1
            for j in range(N // MMF):
                sq2 = sb.tile([2, MMF], F32, tag="sq2")
                nc.scalar.activation(
                    sq2, stage[:, G + j * MMF : G + (j + 1) * MMF], ACT.Square)
                mm = psp.tile([G, MMF], F32, tag="mm")
                nc.tensor.matmul(
                    mm, lhsT=stage[:, 0:G],
                    rhs=stage[:, G + j * MMF : G + (j + 1) * MMF],
                    start=True, stop=False,
                )
                nc.tensor.matmul(
                    mm, lhsT=negh, rhs=sq2,
                    start=False, stop=True,
                )
                nc.vector.tensor_reduce(
                    m16[:, j * (MMF // CW) : (j + 1) * (MMF // CW)],
                    mm.rearrange("g (c w) -> g c w", w=CW),
                    axis=AX.X, op=AL.max,
                )
            if taps:
                nc.sync.dma_start(taps["dbg_m16"].ap()[b], m16)

            # ---- cascade 1 (top-NSEL chunks) with per-iteration gathers ----
            # Each max_index yields 8 chunk ids; their 256B block fetches are
            # issued immediately so the Pool engine works under the cascade.
            ci16 = sb.tile([G, NSEL], U16)
            ci32 = sb.tile([G, NSEL], U32)
            cand = sb.tile([G, NSEL, 4 * CW], F32)
            do_gather = stop_after >= 2
            for i in range(NIT1):
                v8 = sb.tile([G, 8], F32, tag="v8")
                nc.vector.max(v8, m16)
                nc.vector.max_index(ci16[:, i * 8 : (i + 1) * 8], v8, m16)
                if i < NIT1 - 1:
                    nc.vector.match_replace(m16, v8, m16, NEG)
                if do_gather:
                    nc.scalar.copy(ci32[:, i * 8 : (i + 1) * 8],
                                   ci16[:, i * 8 : (i + 1) * 8])
                    for k_ in range(8):
                        s_ = i * 8 + k_
                        nc.gpsimd.indirect_dma_start(
                            out=cand[:, s_, :], out_offset=None,
                            in_=cc_rows,
                            in_offset=IndirectOffsetOnAxis(
                                ap=ci32[:, s_ : s_ + 1], axis=0),
                            element_offset=b * NCH * 4 * CW,
                        )
            if taps:
                nc.sync.dma_start(taps["dbg_ci16"].ap()[b], ci16)
                if stop_after >= 2:
                    nc.sync.dma_start(taps["dbg_cand"].ap()[b], cand)
            per_b[b] = dict(gt_sb=gt_sb, x1=x1, x2=x2, y1=y1, y2=y2,
                            area_a=area_a, ci16=ci16, cand=cand)

        for b in range(BPC):
            if stop_after < 3:
                continue
            d = per_b[b]
            gt_sb, x1, x2, y1, y2 = d["gt_sb"], d["x1"], d["x2"], d["y1"], d["y2"]
            area_a, ci16, cand = d["area_a"], d["ci16"], d["cand"]
            gx = gt_sb[:, 0:1]
            gy = gt_sb[:, 1:2]
            cx_c = cand[:, :, 0 * CW : 1 * CW]
            cy_c = cand[:, :, 1 * CW : 2 * CW]
            w_c = cand[:, :, 2 * CW : 3 * CW]
            h_c = cand[:, :, 3 * CW : 4 * CW]

            # ---------------- exact negated d2 ----------------
            T1 = cw.tile([G, CAND], F32, tag="T1")
            T2 = cw.tile([G, CAND], F32, tag="T2")
            T3 = cw.tile([G, CAND], F32, tag="T3")
            T4 = cw.tile([G, CAND], F32, tag="T4")
            T5 = cw.tile([G, CAND], F32, tag="T5")
            T6 = cw.tile([G, CAND], F32, tag="T6")
            T7 = cw.tile([G, CAND], F32, tag="T7")
            T8 = cw.tile([G, CAND], F32, tag="T8")

            nc.vector.tensor_scalar_sub(T1, cx_c, gx)        # dx
            nc.scalar.activation(T1, T1, ACT.Square)
            nc.vector.tensor_scalar_sub(T2, cy_c, gy)        # dy
            nc.scalar.activation(T2, T2, ACT.Square)
            uc = T3
            nc.vector.scalar_tensor_tensor(
                uc, in0=T1, scalar=-1.0, in1=T2, op0=AL.mult, op1=AL.subtract
            )
            if taps:
                nc.sync.dma_start(taps["dbg_uc"].ap()[b], uc)

            # ---------------- cascade 2: exact top-64 ----------------
            pos16 = sb.tile([G, K], U16)
            for i in range(NIT2):
                v8b = sb.tile([G, 8], F32, tag="v8b")
                nc.vector.max(v8b, uc)
                nc.vector.max_index(pos16[:, i * 8 : (i + 1) * 8], v8b, uc)
                nc.vector.match_replace(uc, v8b, uc, NEG)
            sel = T4
            nc.vector.tensor_scalar(sel, uc, NEG, None, op0=AL.is_equal)
            if taps:
                nc.sync.dma_start(taps["dbg_pos"].ap()[b], pos16)
                nc.sync.dma_start(taps["dbg_sel"].ap()[b], sel)

            if stop_after < 4:
                continue
            # ---------------- compact-space epilogue ----------------
            # T1=kx1 T2=kx2 T5=ky1 T6=ky2
            nc.vector.scalar_tensor_tensor(
                T1, in0=w_c, scalar=-0.5, in1=cx_c, op0=AL.mult, op1=AL.add)
            nc.vector.scalar_tensor_tensor(
                T2, in0=w_c, scalar=0.5, in1=cx_c, op0=AL.mult, op1=AL.add)
            nc.vector.scalar_tensor_tensor(
                T5, in0=h_c, scalar=-0.5, in1=cy_c, op0=AL.mult, op1=AL.add)
            nc.vector.scalar_tensor_tensor(
                T6, in0=h_c, scalar=0.5, in1=cy_c, op0=AL.mult, op1=AL.add)

            nc.vector.tensor_sub(T7, T2, T1)                 # abw
            nc.vector.tensor_sub(T8, T6, T5)                 # abh
            nc.vector.tensor_mul(T7, T7, T8)                 # area_b

            nc.vector.tensor_scalar(T8, T1, x1, None, op0=AL.max)   # ltx
            nc.vector.tensor_scalar(T1, T2, x2, None, op0=AL.min)   # rbx (kx1 dead)
            nc.vector.tensor_sub(T2, T1, T8)                 # wx (kx2 dead)
            nc.vector.tensor_scalar(T2, T2, 0.0, None, op0=AL.max)
            nc.vector.tensor_scalar(T8, T5, y1, None, op0=AL.max)   # lty (ltx dead)
            nc.vector.tensor_scalar(T5, T6, y2, None, op0=AL.min)   # rby (ky1 dead)
            nc.vector.tensor_sub(T6, T5, T8)                 # wy (ky2 dead)
            nc.vector.tensor_scalar(T6, T6, 0.0, None, op0=AL.max)
            nc.vector.tensor_mul(T1, T2, T6)                 # inter (rbx dead)

            nc.vector.scalar_tensor_tensor(
                T2, in0=T7, scalar=area_a, in1=T1,
                op0=AL.add, op1=AL.subtract,
            )                                                # union (wx dead)
            nc.vector.reciprocal(T5, T2)                     # runion (rby dead)
            iou_c = T2
            nc.vector.tensor_mul(iou_c, T1, T5)              # iou (union dead)
            if taps:
                nc.sync.dma_start(taps["dbg_iouc"].ap()[b], iou_c)

            # threshold = mean + std(ddof=1) over the selected 64
            nc.vector.tensor_mul(T1, iou_c, sel)             # iou*sel (inter dead)
            msum = sb.tile([G, 1], F32, tag="msum")
            nc.vector.tensor_reduce(msum, T1, axis=AX.X, op=AL.add)
            mean = sb.tile([G, 1], F32, tag="mean")
            nc.vector.tensor_scalar_mul(mean, msum, 1.0 / K)
            nc.vector.tensor_scalar_sub(T1, iou_c, mean)     # cen
            nc.scalar.activation(T1, T1, ACT.Square)
            nc.vector.tensor_mul(T1, T1, sel)
            vsum = sb.tile([G, 1], F32, tag="vsum")
            nc.vector.tensor_reduce(vsum, T1, axis=AX.X, op=AL.add)
            var = sb.tile([G, 1], F32, tag="var")
            nc.vector.tensor_scalar_mul(
                var, vsum, float(np.float32(1.0) / np.float32(K - 1)))
            std = sb.tile([G, 1], F32, tag="std")
            nc.scalar.activation(std, var, ACT.Sqrt)
            thr = sb.tile([G, 1], F32, tag="thr")
            nc.scalar.activation(thr, std, ACT.Identity, bias=mean, scale=1.0)

            # mask = (iou >= thr) & inside (compact space), STT-chained
            mask_c = T5
            nc.vector.tensor_scalar(mask_c, cx_c, x1, None, op0=AL.is_ge)
            nc.vector.scalar_tensor_tensor(
                T1, in0=cx_c, scalar=x2, in1=mask_c, op0=AL.is_le, op1=AL.mult)
            nc.vector.scalar_tensor_tensor(
                mask_c, in0=cy_c, scalar=y1, in1=T1, op0=AL.is_ge, op1=AL.mult)
            nc.vector.scalar_tensor_tensor(
                T1, in0=cy_c, scalar=y2, in1=mask_c, op0=AL.is_le, op1=AL.mult)
            nc.vector.scalar_tensor_tensor(
                mask_c, in0=iou_c, scalar=thr, in1=T1, op0=AL.is_ge, op1=AL.mult)

            if stop_after < 5:
                continue
            # ---------------- pack original index + mask ----------------
            oidx = cw.tile([G, CAND], U16, tag="t_oidx")
            for e in range(CW):
                nc.scalar.activation(oidx[:, e::CW], ci16, ACT.Copy, scale=16.0)
            nc.vector.tensor_tensor(oidx, oidx, io_e, op=AL.add)
            m16b = cw.tile([G, CAND], U16, tag="t_m16b")
            nc.scalar.activation(m16b, mask_c, ACT.Copy, scale=16384.0)
            nc.vector.tensor_tensor(oidx, oidx, m16b, op=AL.add)
            if taps:
                nc.sync.dma_start(taps["dbg_oidx"].ap()[b], oidx)

            if stop_after < 6:
                continue
            # ---------------- rank map + rank-ordered extraction ----------
            pos_i = sb.tile([G, K], I16, tag="pos_i")
            nc.vector.tensor_copy(pos_i, pos16)
            rkm = cw.tile([G, CAND], I16, tag="t_rkm")
            nc.gpsimd.local_scatter(rkm, io_r, pos_i, channels=G,
                                    num_elems=CAND, num_idxs=K)
            nc.vector.tensor_scalar(rkm, rkm, 1, None, op0=AL.subtract)
            if taps:
                nc.sync.dma_start(taps["dbg_rkm"].ap()[b], rkm)

            sortidx = sb.tile([G, K], U16, tag="sortidx")
            nc.gpsimd.local_scatter(sortidx, oidx, rkm, channels=G,
                                    num_elems=K, num_idxs=CAND)
            iou_u = iou_c.bitcast(U16)
            lo = cw.tile([G, CAND], U16, tag="t_lo")
            nc.vector.tensor_copy(lo, iou_u[:, 0::2])
            hi = cw.tile([G, CAND], U16, tag="t_hi")
            nc.vector.tensor_copy(hi, iou_u[:, 1::2])
            # (kept on DVE: ACT converts through fp32 and would corrupt raw u16 halves)
            slo = sb.tile([G, K], U16, tag="slo")
            nc.gpsimd.local_scatter(slo, lo, rkm, channels=G,
                                    num_elems=K, num_idxs=CAND)
            shi = sb.tile([G, K], U16, tag="shi")
            nc.gpsimd.local_scatter(shi, hi, rkm, channels=G,
                                    num_elems=K, num_idxs=CAND)

            # ---------------- outputs ----------------
            kidx16 = sb.tile([G, K], U16, tag="kidx16")
            nc.vector.tensor_scalar(kidx16, sortidx, 0x3FFF, None,
                                    op0=AL.bitwise_and)
            kidxi = sb.tile([G, K], I32, tag="kidxi")
            nc.vector.tensor_copy(kidxi, kidx16)
            nc.sync.dma_start(out_kidx.ap()[b], kidxi)

            msk16 = sb.tile([G, K], U16, tag="msk16")
            nc.vector.tensor_scalar(msk16, sortidx, 14, None,
                                    op0=AL.logical_shift_right)
            msk8 = sb.tile([G, K], U8, tag="msk8")
            nc.vector.tensor_copy(msk8, msk16)
            nc.sync.dma_start(out_mask.ap()[b], msk8)

            iou_o = sb.tile([G, K], F32, tag="iou_o")
            iou_o_u = iou_o.bitcast(U16)
            nc.vector.tensor_copy(iou_o_u[:, 0::2], slo)
            nc.vector.tensor_copy(iou_o_u[:, 1::2], shi)
            nc.sync.dma_start(out_ious.ap()[b], iou_o)


_BUILT = None


def _shard_inputs(pred_boxes, gt_boxes):
    pred_boxes = np.asarray(pred_boxes, dtype=np.float32)
    gt_boxes = np.asarray(gt_boxes, dtype=np.float32)
    in_maps = []
    for c in range(NCORES):
        pb = pred_boxes[c * BPC : (c + 1) * BPC]
        gb = gt_boxes[c * BPC : (c + 1) * BPC]
        ct = np.concatenate(
            [gb[:, :, :2].transpose(0, 2, 1), pb[:, :, :2].transpose(0, 2, 1)],
            axis=2,
        )
        # chunk-major interleave: [BPC, NCH, 4, CW] with planes cx, cy, w, h
        cc = pb.reshape(BPC, NCH, CW, 4).transpose(0, 1, 3, 2)
        in_maps.append({
            "ct_in": np.ascontiguousarray(ct),
            "pred_cc": np.ascontiguousarray(cc),
            "gt_boxes": np.ascontiguousarray(gb),
        })
    return in_maps


def _assemble(results):
    ious = np.concatenate([results[c]["out_ious"] for c in range(NCORES)], axis=0)
    mask = np.concatenate([results[c]["out_mask"] for c in range(NCORES)], axis=0)
    kidx = np.concatenate([results[c]["out_kidx"] for c in range(NCORES)], axis=0)
    return (
        ious.astype(np.float32),
        mask.astype(bool),
        kidx.astype(np.int32),
    )


def kernel(pred_boxes, gt_boxes):
    global _BUILT
    from concourse.bass_utils import run_bass_kernel_spmd

    if _BUILT is None:
        _BUILT = build_program(NCORES)
    in_maps = _shard_inputs(pred_boxes, gt_boxes)
    res = run_bass_kernel_spmd(_BUILT, in_maps, core_ids=list(range(NCORES)))
    return _assemble(res.results)
